# revision 1
# baseline (speedup 1.0000x reference)
"""Trainium2 Bass kernel for nn_MultiHeadAttention_60078002536549.

Dense transformer block:
    att  = softmax(Q K^T / sqrt(64)) V          (B=2, H=16, N=2048, HD=64)
    x1   = x + att_concat                        (B, N, D=1024)
    out  = x1 + gelu(LN(x1) @ w1 + b1) @ w2 + b2 (FF=4096)

Sharding: tokens are sharded across the 8 cores (data parallel on B and on
the query/token dimension: core i handles batch i//4, token rows
[512*(i%4), 512*(i%4+1))).  Each core loads the full K/V of its batch and
the full FFN weights; no collectives are needed and each core produces its
own 512x1024 slice of the output.

On-chip layout is "feature-major" (transposed): activations live as
[feature partitions, token free-dim] so that
  - S^T = K_dmaj^T @ Q_dmaj needs no transposition of the score matrix,
  - softmax denominators come from a ones-column appended to V,
  - the FFN weights are used in their natural DRAM layout.
Matmuls run as float32r (full PE rate at free-dim 512).
"""

import sys

for _p in ("/opt/trn_rl_repo",):
    if _p not in sys.path:
        sys.path.insert(0, _p)

import numpy as np

import concourse.bass as bass
import concourse.mybir as mybir
import concourse.tile as tile
from concourse.bass import ts
from concourse.bass_utils import run_bass_kernel_spmd
from concourse.masks import make_identity

F32 = mybir.dt.float32
F32R = mybir.dt.float32r
BF16 = mybir.dt.bfloat16
F16 = mybir.dt.float16
AF = mybir.ActivationFunctionType

B, H, N, HD, D, FF = 2, 16, 2048, 64, 1024, 4096
NCORES = 8
TOK = (B * N) // NCORES          # 512 tokens per core
SCALE = float(1.0 / np.sqrt(HD))
EPS = 1e-5

NTT = TOK // 128                 # 4 token sub-tiles per core
KC = N // 128                    # 16 k-token chunks
DC = D // 128                    # 8 feature chunks
FC = FF // 128                   # 32 hidden chunks
NPAIR = H // 2                   # 8 head pairs


def r32(ap):
    return ap.bitcast(F32R)


def build_program(split_waits=True):
    nc = bass.Bass()

    xs = nc.declare_dram_parameter("xs", [TOK, D], F32, isOutput=False)
    qs = nc.declare_dram_parameter("qs", [H, TOK, HD], F32, isOutput=False)
    ks = nc.declare_dram_parameter("ks", [H, N, HD], F32, isOutput=False)
    vs = nc.declare_dram_parameter("vs", [H, N, HD], F32, isOutput=False)
    w1 = nc.declare_dram_parameter("w1", [D, FF], F32, isOutput=False)
    b1 = nc.declare_dram_parameter("b1", [FF], F32, isOutput=False)
    w2 = nc.declare_dram_parameter("w2", [FF, D], F32, isOutput=False)
    b2 = nc.declare_dram_parameter("b2", [D], F32, isOutput=False)
    lnw = nc.declare_dram_parameter("lnw", [D], F32, isOutput=False)
    lnb = nc.declare_dram_parameter("lnb", [D], F32, isOutput=False)
    y = nc.declare_dram_parameter("y", [TOK, D], F32, isOutput=True)

    # DRAM views.  Local token ell = 4*p + tt lives at (partition p,
    # sub-tile tt); every feature-major tile uses free index tt*128 + p.
    xs_v = xs[:].rearrange("(p tt) d -> p tt d", tt=NTT)       # [128, 4, D]
    y_v = y[:].rearrange("(p tt) d -> p tt d", tt=NTT)         # [128, 4, D]
    # w1 sliced as [128 d-rows, dc, 128 ff-cols]; w2 as [128 ff-rows, fc, ...]
    w1_v = w1[:].rearrange("(dc p) f -> p dc f", p=128)        # [128, 8, FF]
    w2_v = w2[:].rearrange("(fc p) d -> p fc d", p=128)        # [128, 32, D]

    with tile.TileContext(nc) as tc:
        build_tile_kernel(nc, tc, xs_v, qs, ks, vs, w1_v, b1, w2_v, b2,
                          lnw, lnb, y_v)
    if split_waits:
        _split_matmul_waits(nc)
    return nc


def _split_matmul_waits(nc):
    """This walrus build accepts only one sync wait per compute engine
    instruction; move extra waits onto a NoOp inserted right before it on
    the same engine.  DMA/queue instructions are left untouched."""
    for f in nc.m.functions:
        for blk in f.blocks:
            new = []
            for inst in blk.instructions:
                si = inst.sync_info
                if si is not None and len(si.on_wait) > 1:
                    waits = list(si.on_wait)
                    for w in waits[:-1]:
                        new.append(mybir.InstNoOp(
                            name=f"waitsplit_{nc.next_id()}",
                            engine=inst.engine, ins=[], outs=[],
                            sync_info=mybir.SyncInfo(on_wait=[w],
                                                     on_update=[])))
                    inst.sync_info = mybir.SyncInfo(
                        on_wait=waits[-1:], on_update=list(si.on_update))
                new.append(inst)
            blk.instructions[:] = new


def build_tile_kernel(nc, tc, xs_v, qs, ks, vs, w1_v, b1, w2_v, b2,
                      lnw, lnb, y_v):
    from contextlib import ExitStack

    est = ExitStack()
    singles = est.enter_context(tc.tile_pool(name="singles", bufs=1))
    persist = est.enter_context(tc.tile_pool(name="persist", bufs=1))

    # ---- constants ----
    ident = singles.tile([128, 128], F32, tag="ident")
    make_identity(nc, ident)
    ident_h = singles.tile([128, 128], BF16, tag="ident_h")
    make_identity(nc, ident_h)
    ones_f32 = singles.tile([128, KC], F32, tag="ones_f32")
    nc.vector.memset(ones_f32, 1.0)
    ones_col = singles.tile([128, 1], F32R, tag="ones_col")
    nc.vector.tensor_copy(out=ones_col, in_=ones_f32[:, 0:1])
    eps_t = singles.tile([1, 1], F32, tag="eps")
    nc.vector.memset(eps_t, EPS)

    lnw_sb = singles.tile([128, DC], F32, tag="lnw")
    nc.sync.dma_start(out=lnw_sb, in_=lnw[:].rearrange("(c p) -> p c", p=128))
    lnb_sb = singles.tile([128, DC], F32, tag="lnb")
    nc.sync.dma_start(out=lnb_sb, in_=lnb[:].rearrange("(c p) -> p c", p=128))
    b2_sb = singles.tile([128, DC], F32, tag="b2")
    nc.sync.dma_start(out=b2_sb, in_=b2[:].rearrange("(c p) -> p c", p=128))
    b1_sb = singles.tile([128, FC], F32, tag="b1")
    nc.sync.dma_start(out=b1_sb, in_=b1[:].rearrange("(c p) -> p c", p=128))

    # ---- x load + transpose to feature-major ----
    # xT[j] : [128 dpart, 4 tt, 128 p] ; x1T[j] same shape (post-residual)
    xT = [persist.tile([128, NTT, 128], F32, name=f"xT{j}", tag=f"xT{j}") for j in range(DC)]
    x1T = [persist.tile([128, NTT, 128], F32R, name=f"x1T{j}", tag=f"x1T{j}") for j in range(DC)]

    with tc.tile_pool(name="xstage", bufs=1) as xsp, \
         tc.tile_pool(name="xtp", bufs=2, space="PSUM") as xtp:
        x_stage = xsp.tile([128, NTT, D], F32, tag="xstage")
        nc.sync.dma_start(out=x_stage, in_=xs_v)
        for j in range(DC):
            tpx = xtp.tile([128, NTT, 128], F32, tag="tpx")
            for tt in range(NTT):
                nc.tensor.transpose(tpx[:, tt, :],
                                    x_stage[:, tt, ts(j, 128)], ident)
            nc.vector.tensor_copy(out=xT[j], in_=tpx)

    # =================== attention ===================
    att_est = ExitStack()
    kst_p = att_est.enter_context(tc.tile_pool(name="kst", bufs=2))
    qst_p = att_est.enter_context(tc.tile_pool(name="qst", bufs=2))
    vst_p = att_est.enter_context(tc.tile_pool(name="vst", bufs=2))
    vaug_p = att_est.enter_context(tc.tile_pool(name="vaug", bufs=2))
    kpair_p = att_est.enter_context(tc.tile_pool(name="kpair", bufs=2))
    qpair_p = att_est.enter_context(tc.tile_pool(name="qpair", bufs=2))
    exp_p = att_est.enter_context(tc.tile_pool(name="expp", bufs=3))
    nrm_p = att_est.enter_context(tc.tile_pool(name="nrm", bufs=2))
    bcd_p = att_est.enter_context(tc.tile_pool(name="bcd", bufs=2, space="DRAM"))
    sq_p = att_est.enter_context(tc.tile_pool(name="sqp", bufs=2))
    s_ps = att_est.enter_context(tc.tile_pool(name="s_ps", bufs=2, space="PSUM"))
    att_ps = att_est.enter_context(tc.tile_pool(name="att_ps", bufs=1, space="PSUM"))

    st_est = ExitStack()
    st_ps = st_est.enter_context(tc.tile_pool(name="st_ps", bufs=1, space="PSUM"))
    stats = st_ps.tile([1, 2, TOK], F32, tag="stats")

    def emit_stats(jprev, sq_tile):
        x1v = x1T[jprev].rearrange("p tt f -> p (tt f)")
        nc.tensor.matmul(stats[:, 0, :], ones_col, x1v,
                         start=(jprev == 0), stop=(jprev == NPAIR - 1))
        nc.tensor.matmul(stats[:, 1, :], ones_col, sq_tile,
                         start=(jprev == 0), stop=(jprev == NPAIR - 1))

    pend_sq = None
    for j in range(NPAIR):
        ha, hb = 2 * j, 2 * j + 1
        if pend_sq is not None:
            sq_prev = sq_p.tile([128, TOK], F32R, tag="sq")
            nc.scalar.activation(
                sq_prev, x1T[j - 1].rearrange("p tt f -> p (tt f)"), AF.Square)
            pend_sq = sq_prev
        # ---- stage K/V (token = 16*p + c), Q (token = 4*p + tt) ----
        ka_st = kst_p.tile([128, KC, HD], BF16, tag="ka")
        kb_st = kst_p.tile([128, KC, HD], BF16, tag="kb")
        nc.gpsimd.dma_start(out=ka_st, in_=ks[ha].rearrange("(p c) d -> p c d", p=128))
        nc.gpsimd.dma_start(out=kb_st, in_=ks[hb].rearrange("(p c) d -> p c d", p=128))
        va_st = vst_p.tile([128, KC, HD], F32, tag="va")
        vb_st = vst_p.tile([128, KC, HD], F32, tag="vb")
        nc.sync.dma_start(out=va_st, in_=vs[ha].rearrange("(p c) d -> p c d", p=128))
        nc.sync.dma_start(out=vb_st, in_=vs[hb].rearrange("(p c) d -> p c d", p=128))
        qa_st = qst_p.tile([128, NTT, HD], BF16, tag="qa")
        qb_st = qst_p.tile([128, NTT, HD], BF16, tag="qb")
        nc.gpsimd.dma_start(out=qa_st, in_=qs[ha].rearrange("(p t) d -> p t d", p=128))
        nc.gpsimd.dma_start(out=qb_st, in_=qs[hb].rearrange("(p t) d -> p t d", p=128))

        # ---- V augmented with a ones column (softmax denominator) ----
        va = vaug_p.tile([128, KC, HD + 1], F16, tag="vaug_a")
        vb = vaug_p.tile([128, KC, HD + 1], F16, tag="vaug_b")
        nc.vector.tensor_copy(out=va[:, :, HD:HD + 1], in_=ones_f32)
        nc.vector.tensor_copy(out=vb[:, :, HD:HD + 1], in_=ones_f32)
        nc.vector.tensor_copy(out=va[:, :, 0:HD], in_=va_st)
        nc.vector.tensor_copy(out=vb[:, :, 0:HD], in_=vb_st)

        # ---- pair the two heads side by side in the free dim, then
        # transpose [128 tok, 128] -> [128 (a-dims | b-dims), 128 tok].
        kpd = kpair_p.tile([128, KC, 128], BF16, tag="kpd")
        nc.vector.tensor_copy(out=kpd[:, :, 0:HD], in_=ka_st)
        nc.vector.tensor_copy(out=kpd[:, :, HD:128], in_=kb_st)
        qpd = qpair_p.tile([128, NTT, 128], BF16, tag="qpd")
        nc.vector.tensor_copy(out=qpd[:, :, 0:HD], in_=qa_st)
        nc.vector.tensor_copy(out=qpd[:, :, HD:128], in_=qb_st)
        kpg = []
        for g in range(KC // 4):
            tpk = s_ps.tile([128, 4, 128], BF16, tag="s")
            for i in range(4):
                nc.tensor.transpose(tpk[:, i, :], kpd[:, 4 * g + i, :], ident_h)
            kg = kpair_p.tile([128, 4, 128], BF16, name=f"kp{g}", tag=f"kp{g}")
            nc.vector.tensor_copy(out=kg, in_=tpk)
            kpg.append(kg)
        qp = qpair_p.tile([128, NTT, 128], BF16, tag="qp")
        tpq = s_ps.tile([128, NTT, 128], BF16, tag="s")
        for tt in range(NTT):
            nc.tensor.transpose(tpq[:, tt, :], qpd[:, tt, :], ident_h)
        nc.vector.tensor_copy(out=qp, in_=tpq)

        # ---- scores + softmax-exp + AV accumulation ----
        att_a = att_ps.tile([HD + 1, TOK], F32, tag="att_a")
        att_b = att_ps.tile([HD + 1, TOK], F32, tag="att_b")

        def exp_av(s, c):
            e = exp_p.tile([128, 2, TOK], F16, tag="e")
            nc.scalar.activation(e, s, AF.Exp, scale=SCALE)
            nc.tensor.matmul(att_a, va[:, c, :], e[:, 0, :],
                             start=(c == 0), stop=(c == KC - 1))
            nc.tensor.matmul(att_b, vb[:, c, :], e[:, 1, :],
                             start=(c == 0), stop=(c == KC - 1))

        pend = None
        for c in range(KC):
            s = s_ps.tile([128, 2, TOK], F32, tag="s")
            nc.tensor.matmul(s[:, 0, :], kpg[c // 4][0:64, c % 4, :],
                             qp[0:64, :, :], tile_position=(0, 0))
            nc.tensor.matmul(s[:, 1, :], kpg[c // 4][64:128, c % 4, :],
                             qp[64:128, :, :], tile_position=(64, 0))
            if pend is not None:
                exp_av(*pend)
            pend = (s, c)
        exp_av(*pend)

        # ---- normalize by the ones-row sums, add x residual ----
        scr = nrm_p.tile([128, 2, TOK], F32, tag="scr")
        nc.scalar.activation(scr[64:65, 0, :], att_a[HD:HD + 1, :], AF.Ln)
        nc.scalar.activation(scr[64:65, 1, :], att_b[HD:HD + 1, :], AF.Ln)
        nc.scalar.activation(scr[64:65, :, :], scr[64:65, :, :], AF.Exp,
                             scale=-1.0)
        cpa = nrm_p.tile([64, TOK], F32, tag="cpa")
        cpb = nrm_p.tile([64, TOK], F32, tag="cpb")
        nc.vector.tensor_copy(out=cpa, in_=att_a[0:HD, :])
        nc.vector.tensor_copy(out=cpb, in_=att_b[0:HD, :])
        bcd = bcd_p.tile([2, TOK], F32, tag="bcd")
        nc.sync.dma_start(out=bcd, in_=scr[64:65, :, :])
        bca = nrm_p.tile([64, TOK], F32, tag="bca")
        bcb = nrm_p.tile([64, TOK], F32, tag="bcb")
        nc.sync.dma_start(out=bca, in_=bcd[0:1, :].to_broadcast((64, TOK)))
        nc.sync.dma_start(out=bcb, in_=bcd[1:2, :].to_broadcast((64, TOK)))
        natt = nrm_p.tile([128, NTT, 128], F32, tag="natt")
        tmpb = nrm_p.tile([64, TOK], F32, tag="tmpb")
        nav = natt.rearrange("p tt f -> p (tt f)")
        nc.vector.tensor_mul(nav[0:64, :], cpa, bca)
        nc.vector.tensor_mul(tmpb, cpb, bcb)
        nc.sync.dma_start(out=nav[64:128, :], in_=tmpb)
        nc.vector.tensor_add(x1T[j], natt, xT[j])

        # flush the previous pair's layer-norm stats (deferred so the PE
        # queue at the pair boundary is not blocked behind the ACT square)
        if pend_sq is not None:
            emit_stats(j - 1, pend_sq)
        pend_sq = True

    sq_last = sq_p.tile([128, TOK], F32R, tag="sq")
    nc.scalar.activation(
        sq_last, x1T[NPAIR - 1].rearrange("p tt f -> p (tt f)"), AF.Square)
    emit_stats(NPAIR - 1, sq_last)

    # ---- layer-norm scalars (still inside the attention pool scope so the
    # stats PSUM bank can be read before those pools close) ----
    mu = persist.tile([1, TOK], F32, tag="mu")
    msq = persist.tile([1, TOK], F32, tag="msq")
    var = persist.tile([1, TOK], F32, tag="var")
    rstd = persist.tile([1, TOK], F32, tag="rstd")
    nc.vector.tensor_scalar_mul(mu, stats[:, 0, :], 1.0 / D)
    nc.vector.tensor_scalar_mul(msq, stats[:, 1, :], 1.0 / D)
    st_est.close()
    nc.vector.tensor_mul(var, mu, mu)
    nc.vector.tensor_sub(var, msq, var)
    # rstd = exp(-0.5 * ln(var + eps)) -- stays within the ln/exp table set
    nc.scalar.activation(var, var, AF.Ln, bias=eps_t)
    nc.scalar.activation(rstd, var, AF.Exp, scale=-0.5)

    mu_b = persist.tile([128, TOK], F32, tag="mu_b")
    rstd_b = persist.tile([128, TOK], F32, tag="rstd_b")
    lnd = bcd_p.tile([2, TOK], F32, tag="lnd")
    nc.sync.dma_start(out=lnd[0:1, :], in_=mu)
    nc.sync.dma_start(out=lnd[1:2, :], in_=rstd)
    nc.sync.dma_start(out=mu_b, in_=lnd[0:1, :].to_broadcast((128, TOK)))
    nc.sync.dma_start(out=rstd_b, in_=lnd[1:2, :].to_broadcast((128, TOK)))

    att_est.close()

    # =================== FFN scope ===================
    ffn_est = ExitStack()
    mm_ps = ffn_est.enter_context(tc.tile_pool(name="mm_ps", bufs=4, space="PSUM"))
    ln_p = ffn_est.enter_context(tc.tile_pool(name="ln", bufs=2))
    w1_p = ffn_est.enter_context(tc.tile_pool(name="w1p", bufs=3))
    w2_p = ffn_est.enter_context(tc.tile_pool(name="w2p", bufs=2))
    g_p = ffn_est.enter_context(tc.tile_pool(name="gp", bufs=1))
    h_p = ffn_est.enter_context(tc.tile_pool(name="hp", bufs=1))
    o_p = ffn_est.enter_context(tc.tile_pool(name="op", bufs=1))
    onat_p = ffn_est.enter_context(tc.tile_pool(name="onat", bufs=2))
    otp_ps = ffn_est.enter_context(tc.tile_pool(name="otp", bufs=1, space="PSUM"))

    hT = [h_p.tile([128, NTT, 128], F16, name=f"hT{j}", tag=f"hT{j}") for j in range(DC)]
    for j in range(DC):
        x1v = x1T[j].rearrange("p tt f -> p (tt f)")
        hv = hT[j].rearrange("p tt f -> p (tt f)")
        t = ln_p.tile([128, TOK], F32, tag="lnt")
        nc.vector.tensor_sub(t, x1v, mu_b)
        nc.vector.tensor_mul(t, t, rstd_b)
        nc.scalar.activation(hv, t, AF.Identity,
                             bias=lnb_sb[:, j:j + 1],
                             scale=lnw_sb[:, j:j + 1])

    # =================== FFN ===================
    gT = [g_p.tile([128, TOK], F16, name=f"gT{f}", tag=f"gT{f}") for f in range(FC)]
    for f in range(FC):
        w1s = w1_p.tile([128, DC, 128], F32, tag="w1s")
        nc.sync.dma_start(out=w1s[:, 0:DC // 2, :],
                          in_=w1_v[:, 0:DC // 2, ts(f, 128)])
        nc.scalar.dma_start(out=w1s[:, DC // 2:, :],
                            in_=w1_v[:, DC // 2:, ts(f, 128)])
        w1t = w1_p.tile([128, DC, 128], F16, tag="w1t")
        nc.vector.tensor_copy(out=w1t, in_=w1s)
        ps = mm_ps.tile([128, TOK], F32, tag="mm")
        for dc in range(DC):
            nc.tensor.matmul(ps, w1t[:, dc, :],
                             hT[dc].rearrange("p tt f -> p (tt f)"),
                             start=(dc == 0), stop=(dc == DC - 1))
        nc.scalar.activation(gT[f], ps, AF.Gelu, bias=b1_sb[:, f:f + 1])

    outT = [o_p.tile([128, NTT, 128], F32, name=f"outT{d}", tag=f"outT{d}") for d in range(DC)]
    for dd in range(DC):
        w2s = w2_p.tile([128, FC, 128], F32, tag="w2s")
        nc.sync.dma_start(out=w2s[:, 0:FC // 2, :],
                          in_=w2_v[:, 0:FC // 2, ts(dd, 128)])
        nc.scalar.dma_start(out=w2s[:, FC // 2:, :],
                            in_=w2_v[:, FC // 2:, ts(dd, 128)])
        w2t = w2_p.tile([128, FC, 128], F16, tag="w2t")
        nc.vector.tensor_copy(out=w2t, in_=w2s)
        ps = mm_ps.tile([128, TOK], F32, tag="mm")
        for fc in range(FC):
            nc.tensor.matmul(ps, w2t[:, fc, :], gT[fc],
                             start=(fc == 0), stop=(fc == FC - 1))
        ov = outT[dd].rearrange("p tt f -> p (tt f)")
        nc.vector.tensor_scalar_add(ov, ps, b2_sb[:, dd:dd + 1])
        nc.vector.tensor_add(ov, ov,
                             x1T[dd].rearrange("p tt f -> p (tt f)"))


    # =================== back to natural layout + store ===================
    for tt in range(NTT):
        otp = otp_ps.tile([128, DC, 128], F32, tag="otp")
        for dd in range(DC):
            nc.tensor.transpose(otp[:, dd, :], outT[dd][:, tt, :], ident)
        onat = onat_p.tile([128, DC, 128], F32, tag="onat")
        nc.vector.tensor_copy(out=onat, in_=otp)
        nc.sync.dma_start(out=y_v[:, tt, :],
                          in_=onat.rearrange("p dd f -> p (dd f)"))

    ffn_est.close()
    est.close()


_PROGRAMS = {}


def get_program(split_waits=True):
    if split_waits not in _PROGRAMS:
        _PROGRAMS[split_waits] = build_program(split_waits)
    return _PROGRAMS[split_waits]


def make_in_maps(x, image_q, image_k, image_v, ln_w, ln_b, w1, b1, w2, b2):
    asf = lambda a: np.ascontiguousarray(np.asarray(a, dtype=np.float32))
    x = asf(x); image_q = asf(image_q); image_k = asf(image_k)
    image_v = asf(image_v)
    shared = {
        "w1": asf(w1), "b1": asf(b1), "w2": asf(w2), "b2": asf(b2),
        "lnw": asf(ln_w), "lnb": asf(ln_b),
    }
    in_maps = []
    for core in range(NCORES):
        b, r = divmod(core, NCORES // B)
        rows = slice(TOK * r, TOK * (r + 1))
        in_maps.append({
            "xs": asf(x[b, rows]),
            "qs": asf(image_q[b, :, rows]),
            "ks": image_k[b],
            "vs": image_v[b],
            **shared,
        })
    return in_maps


def run_cores(in_maps, trace=False, **kw):
    nc = get_program()
    return run_bass_kernel_spmd(nc, in_maps, core_ids=list(range(NCORES)),
                                trace=trace, **kw)


def kernel(x, image_q, image_k, image_v, ln_w, ln_b, w1, b1, w2, b2):
    in_maps = make_in_maps(x, image_q, image_k, image_v, ln_w, ln_b,
                           w1, b1, w2, b2)
    res = run_cores(in_maps)
    out = np.empty((B, N, D), dtype=np.float32)
    for core in range(NCORES):
        b, r = divmod(core, NCORES // B)
        out[b, TOK * r:TOK * (r + 1)] = res.results[core]["y"]
    return out



# revision 6
# speedup vs baseline: 1.2913x; 1.2913x over previous
"""Trainium2 Bass kernel for nn_MultiHeadAttention_60078002536549.

Dense transformer block:
    att  = softmax(Q K^T / sqrt(64)) V          (B=2, H=16, N=2048, HD=64)
    x1   = x + att_concat                        (B, N, D=1024)
    out  = x1 + gelu(LN(x1) @ w1 + b1) @ w2 + b2 (FF=4096)

Sharding: tokens are sharded across the 8 cores (core i handles batch i//4,
token rows [512*(i%4), 512*(i%4+1))).  Each core loads the full K/V of its
batch and the full FFN weights; no collectives.

v2: all layout work happens on the HOST (numpy) —
  - K and Q arrive dim-major and head-paired (two 64-dim heads stacked on
    the partition axis), with the 1/sqrt(64) score scale pre-folded into Q
    (exactly representable: 2^-3), so scores S^T = K_pair^T-free @ Q needs
    zero on-chip transposes and exp() needs no scale argument.
  - V arrives token-major per k-chunk with the softmax-denominator ones
    column pre-appended.
  - ln_w/ln_b are folded into w1/b1 (exact, linear), so the LN apply is
    just (x1-mu)*rstd.
  - w1/w2 arrive bf16 in stationary-operand-ready tiling; x arrives
    feature-major; the output is written feature-major and transposed back
    on the host.
On-chip the attention inner loop is the ACT-bound exp stream with score
and AV matmuls (bf16/f16) overlapped; softmax reciprocals run on the DVE
(reciprocal_approx_fast) to keep ACT for exp only; LN stats accumulate on
the PE via a ones-column matmul, deferred one pair to keep the PE dense.
"""

import sys

for _p in ("/opt/trn_rl_repo",):
    if _p not in sys.path:
        sys.path.insert(0, _p)

import ml_dtypes
import numpy as np

import concourse.bass as bass
import concourse.mybir as mybir
import concourse.tile as tile
from concourse.bass import ts
from concourse.bass_utils import run_bass_kernel_spmd

F32 = mybir.dt.float32
F32R = mybir.dt.float32r
BF16 = mybir.dt.bfloat16
F16 = mybir.dt.float16
AF = mybir.ActivationFunctionType

B, H, N, HD, D, FF = 2, 16, 2048, 64, 1024, 4096
NCORES = 8
TOK = (B * N) // NCORES          # 512 tokens per core
SCALE = float(1.0 / np.sqrt(HD))
EPS = 1e-5

KC = N // 128                    # 16 k-token chunks
DC = D // 128                    # 8 feature chunks
FC = FF // 128                   # 32 hidden chunks
NPAIR = H // 2                   # 8 head pairs


def build_program(split_waits=True):
    nc = bass.Bass()

    kp = nc.declare_dram_parameter("kp", [NPAIR, 128, KC, 128], BF16, isOutput=False)
    qp = nc.declare_dram_parameter("qp", [NPAIR, 128, TOK], BF16, isOutput=False)
    vp = nc.declare_dram_parameter("vp", [NPAIR, 2, 128, KC, HD + 1], F16, isOutput=False)
    xt = nc.declare_dram_parameter("xt", [128, DC, TOK], F32, isOutput=False)
    w1p = nc.declare_dram_parameter("w1p", [FC, 128, DC * 128], BF16, isOutput=False)
    b1p = nc.declare_dram_parameter("b1p", [FF], F32, isOutput=False)
    w2p = nc.declare_dram_parameter("w2p", [DC, 128, FC * 128], BF16, isOutput=False)
    b2p = nc.declare_dram_parameter("b2p", [D], F32, isOutput=False)
    y = nc.declare_dram_parameter("y", [DC, 128, TOK], F32, isOutput=True)

    with tile.TileContext(nc) as tc:
        build_tile_kernel(nc, tc, kp, qp, vp, xt, w1p, b1p, w2p, b2p, y)
    if split_waits:
        _split_matmul_waits(nc)
    return nc


def _split_matmul_waits(nc):
    """This walrus build accepts only one sync wait per compute engine
    instruction; move extra waits onto a NoOp inserted right before it on
    the same engine.  DMA/queue instructions are left untouched."""
    for f in nc.m.functions:
        for blk in f.blocks:
            new = []
            for inst in blk.instructions:
                si = inst.sync_info
                if si is not None and len(si.on_wait) > 1:
                    waits = list(si.on_wait)
                    for w in waits[:-1]:
                        new.append(mybir.InstNoOp(
                            name=f"waitsplit_{nc.next_id()}",
                            engine=inst.engine, ins=[], outs=[],
                            sync_info=mybir.SyncInfo(on_wait=[w],
                                                     on_update=[])))
                    inst.sync_info = mybir.SyncInfo(
                        on_wait=waits[-1:], on_update=list(si.on_update))
                new.append(inst)
            blk.instructions[:] = new


def build_tile_kernel(nc, tc, kp, qp, vp, xt, w1p, b1p, w2p, b2p, y):
    from contextlib import ExitStack

    est = ExitStack()
    singles = est.enter_context(tc.tile_pool(name="singles", bufs=1))
    persist = est.enter_context(tc.tile_pool(name="persist", bufs=1))

    # ---- constants / small weights ----
    ones_f32 = singles.tile([128, 1], F32, tag="ones_f32")
    nc.vector.memset(ones_f32, 1.0)
    ones_col = singles.tile([128, 1], F32R, tag="ones_col")
    nc.vector.tensor_copy(out=ones_col, in_=ones_f32)
    eps_t = singles.tile([1, 1], F32, tag="eps")
    nc.vector.memset(eps_t, EPS)

    b1s = singles.tile([128, FC], F32, tag="b1s")
    nc.scalar.dma_start(out=b1s, in_=b1p[:].rearrange("(c p) -> p c", p=128))
    b2s = singles.tile([128, DC], F32, tag="b2s")
    nc.scalar.dma_start(out=b2s, in_=b2p[:].rearrange("(c p) -> p c", p=128))

    # ---- persistent activations ----
    xta = persist.tile([128, DC, TOK], F32, tag="xta")
    x1 = persist.tile([128, DC, TOK], F32R, tag="x1")
    ht = persist.tile([128, DC, TOK], BF16, tag="ht")
    gt = persist.tile([128, FC, TOK], BF16, tag="gt")
    w1a = persist.tile([128, FC, DC * 128], BF16, tag="w1a")

    # x (feature-major) on the scalar queue, then w1 in 4 slabs so it
    # streams in behind the per-pair K/V loads during attention.
    nc.scalar.dma_start(out=xta, in_=xt[:].rearrange("p dc t -> p (dc t)")
                        .rearrange("p (dc t) -> p dc t", dc=DC))
    W1SLAB = FC // 4
    for sl in range(4):
        nc.scalar.dma_start(
            out=w1a[:, ts(sl, W1SLAB), :],
            in_=w1p[ts(sl, W1SLAB)].rearrange("f p q -> p f q"))

    # =================== attention ===================
    att_est = ExitStack()
    kq_p = att_est.enter_context(tc.tile_pool(name="kq", bufs=2))
    v_p = att_est.enter_context(tc.tile_pool(name="vp", bufs=2))
    e_p = att_est.enter_context(tc.tile_pool(name="ep", bufs=3))
    nrm_p = att_est.enter_context(tc.tile_pool(name="nrm", bufs=2))
    sq_p = att_est.enter_context(tc.tile_pool(name="sqp", bufs=2))
    bcd_p = att_est.enter_context(tc.tile_pool(name="bcd", bufs=2, space="DRAM"))
    s_ps = att_est.enter_context(tc.tile_pool(name="s_ps", bufs=2, space="PSUM"))
    att_ps = att_est.enter_context(tc.tile_pool(name="att_ps", bufs=1, space="PSUM"))

    st_est = ExitStack()
    st_ps = st_est.enter_context(tc.tile_pool(name="st_ps", bufs=1, space="PSUM"))
    stats = st_ps.tile([1, 2, TOK], F32, tag="stats")

    def emit_stats(jprev, sq_tile):
        x1v = x1[:, jprev, :]
        nc.tensor.matmul(stats[:, 0, :], ones_col, x1v,
                         start=(jprev == 0), stop=(jprev == NPAIR - 1))
        nc.tensor.matmul(stats[:, 1, :], ones_col, sq_tile,
                         start=(jprev == 0), stop=(jprev == NPAIR - 1))

    pend_stats = None            # (jprev, sq_tile)
    for j in range(NPAIR):
        # ---- stage K/Q (gpsimd queue), V (sync queue) ----
        kg = kq_p.tile([128, KC, 128], BF16, tag="kg")
        nc.gpsimd.dma_start(out=kg, in_=kp[j])
        qt = kq_p.tile([128, TOK], BF16, tag="qt")
        nc.gpsimd.dma_start(out=qt, in_=qp[j])
        va = v_p.tile([128, KC, HD + 1], F16, tag="va")
        nc.sync.dma_start(out=va, in_=vp[j, 0])
        vb = v_p.tile([128, KC, HD + 1], F16, tag="vb")
        nc.sync.dma_start(out=vb, in_=vp[j, 1])

        att_a = att_ps.tile([HD + 1, TOK], F32, tag="att_a")
        att_b = att_ps.tile([HD + 1, TOK], F32, tag="att_b")

        def exp_av(s, c):
            e = e_p.tile([128, 2, TOK], F16, tag="e")
            nc.scalar.activation(e, s, AF.Exp)
            nc.tensor.matmul(att_a, va[:, c, :], e[:, 0, :],
                             start=(c == 0), stop=(c == KC - 1))
            nc.tensor.matmul(att_b, vb[:, c, :], e[:, 1, :],
                             start=(c == 0), stop=(c == KC - 1))

        pend = None
        for c in range(KC):
            s = s_ps.tile([128, 2, TOK], F32, tag="s")
            nc.tensor.matmul(s[:, 0, :], kg[0:64, c, :], qt[0:64, :],
                             tile_position=(0, 0))
            nc.tensor.matmul(s[:, 1, :], kg[64:128, c, :], qt[64:128, :],
                             tile_position=(64, 0))
            if pend is not None:
                exp_av(*pend)
            pend = (s, c)
            # deferred LN stats for the previous pair, tucked into the
            # second chunk so the PE queue at the pair boundary stays busy
            if c == 2 and pend_stats is not None:
                emit_stats(*pend_stats)
                pend_stats = None
        exp_av(*pend)

        # ---- drain att PSUM to SBUF, reciprocal on DVE, broadcast ----
        cpa = nrm_p.tile([HD + 1, TOK], F32, tag="cpa")
        cpb = nrm_p.tile([HD + 1, TOK], F32, tag="cpb")
        nc.vector.tensor_copy(out=cpa, in_=att_a)
        nc.vector.tensor_copy(out=cpb, in_=att_b)
        rab = nrm_p.tile([1, 2, TOK], F32, tag="rab")
        nc.vector.reciprocal(out=rab[:, 0, :], in_=cpa[HD:HD + 1, :])
        nc.vector.reciprocal(out=rab[:, 1, :], in_=cpb[HD:HD + 1, :])
        bcd = bcd_p.tile([2, TOK], F32, tag="bcd")
        nc.gpsimd.dma_start(out=bcd, in_=rab)
        bca = nrm_p.tile([HD, TOK], F32, tag="bca")
        bcb = nrm_p.tile([HD, TOK], F32, tag="bcb")
        nc.gpsimd.dma_start(out=bca, in_=bcd[0:1, :].to_broadcast((HD, TOK)))
        nc.gpsimd.dma_start(out=bcb, in_=bcd[1:2, :].to_broadcast((HD, TOK)))

        # ---- normalize + residual into x1 (feature block j) ----
        nc.vector.tensor_mul(x1[0:HD, j, :], cpa[0:HD, :], bca)
        nc.vector.tensor_mul(x1[HD:128, j, :], cpb[0:HD, :], bcb)
        nc.vector.tensor_add(x1[0:HD, j, :], x1[0:HD, j, :], xta[0:HD, j, :])
        nc.vector.tensor_add(x1[HD:128, j, :], x1[HD:128, j, :],
                             xta[HD:128, j, :])

        # squares for the LN variance (DVE, off the ACT critical path)
        sq = sq_p.tile([128, TOK], F32R, tag="sq")
        nc.vector.tensor_mul(sq, x1[:, j, :], x1[:, j, :])
        pend_stats = (j, sq)

    emit_stats(*pend_stats)

    # ---- layer-norm scalars ----
    mu = persist.tile([1, TOK], F32, tag="mu")
    msq = persist.tile([1, TOK], F32, tag="msq")
    var = persist.tile([1, TOK], F32, tag="var")
    rstd = persist.tile([1, TOK], F32, tag="rstd")
    nc.vector.tensor_scalar_mul(mu, stats[:, 0, :], 1.0 / D)
    nc.vector.tensor_scalar_mul(msq, stats[:, 1, :], 1.0 / D)
    st_est.close()
    nc.vector.tensor_mul(var, mu, mu)
    nc.vector.tensor_sub(var, msq, var)
    # rstd = exp(-0.5 * ln(var + eps)) -- stays within the ln/exp table set
    nc.scalar.activation(var, var, AF.Ln, bias=eps_t)
    nc.scalar.activation(rstd, var, AF.Exp, scale=-0.5)

    mu_b = persist.tile([128, TOK], F32, tag="mu_b")
    rstd_b = persist.tile([128, TOK], F32, tag="rstd_b")
    lnd = bcd_p.tile([2, TOK], F32, tag="lnd")
    nc.gpsimd.dma_start(out=lnd[0:1, :], in_=mu)
    nc.gpsimd.dma_start(out=lnd[1:2, :], in_=rstd)
    nc.gpsimd.dma_start(out=mu_b, in_=lnd[0:1, :].to_broadcast((128, TOK)))
    nc.gpsimd.dma_start(out=rstd_b, in_=lnd[1:2, :].to_broadcast((128, TOK)))

    att_est.close()

    # =================== FFN ===================
    ffn_est = ExitStack()
    mm_ps = ffn_est.enter_context(tc.tile_pool(name="mm_ps", bufs=6, space="PSUM"))
    ln_p = ffn_est.enter_context(tc.tile_pool(name="ln", bufs=2))
    w2_p = ffn_est.enter_context(tc.tile_pool(name="w2p", bufs=2))
    o_p = ffn_est.enter_context(tc.tile_pool(name="op", bufs=2))

    # LN apply: ht = (x1 - mu) * rstd   (ln_w/ln_b folded into w1/b1)
    for jj in range(DC):
        t = ln_p.tile([128, TOK], F32, tag="lnt")
        nc.vector.tensor_sub(t, x1[:, jj, :], mu_b)
        nc.vector.tensor_mul(ht[:, jj, :], t, rstd_b)

    # FFN1: gt[fc] = gelu(w1[:,fc]^T h + b1[fc])
    for f in range(FC):
        ps = mm_ps.tile([128, TOK], F32, tag="mm")
        for dc in range(DC):
            nc.tensor.matmul(ps, w1a[:, f, ts(dc, 128)], ht[:, dc, :],
                             start=(dc == 0), stop=(dc == DC - 1))
        nc.scalar.activation(gt[:, f, :], ps, AF.Gelu, bias=b1s[:, f:f + 1])

    # FFN2: y[dd] = w2[:,dd]^T g + b2[dd] + x1[dd]
    for dd in range(DC):
        w2t = w2_p.tile([128, FC * 128], BF16, tag="w2t")
        nc.scalar.dma_start(out=w2t, in_=w2p[dd])
        ps = mm_ps.tile([128, TOK], F32, tag="mm")
        for fc in range(FC):
            nc.tensor.matmul(ps, w2t[:, ts(fc, 128)], gt[:, fc, :],
                             start=(fc == 0), stop=(fc == FC - 1))
        o = o_p.tile([128, TOK], F32, tag="o")
        nc.vector.tensor_scalar_add(o, ps, b2s[:, dd:dd + 1])
        nc.vector.tensor_add(o, o, x1[:, dd, :])
        nc.sync.dma_start(out=y[dd], in_=o)

    ffn_est.close()
    est.close()


_PROGRAMS = {}


def get_program(split_waits=True):
    if split_waits not in _PROGRAMS:
        _PROGRAMS[split_waits] = build_program(split_waits)
    return _PROGRAMS[split_waits]


def make_in_maps(x, image_q, image_k, image_v, ln_w, ln_b, w1, b1, w2, b2):
    f32 = np.float32
    bf16 = ml_dtypes.bfloat16
    x = np.asarray(x, f32)
    image_q = np.asarray(image_q, f32)
    image_k = np.asarray(image_k, f32)
    image_v = np.asarray(image_v, f32)
    w1 = np.asarray(w1, f32)
    b1 = np.asarray(b1, f32)
    w2 = np.asarray(w2, f32)
    b2 = np.asarray(b2, f32)
    ln_w = np.asarray(ln_w, f32)
    ln_b = np.asarray(ln_b, f32)

    # fold LN affine into w1/b1 (exact)
    w1f = ln_w[:, None] * w1
    b1f = b1 + ln_b @ w1
    w1p = np.ascontiguousarray(
        w1f.reshape(DC, 128, FC, 128).transpose(2, 1, 0, 3)
        .reshape(FC, 128, DC * 128).astype(bf16))
    w2p = np.ascontiguousarray(
        w2.reshape(FC, 128, DC, 128).transpose(2, 1, 0, 3)
        .reshape(DC, 128, FC * 128).astype(bf16))

    shared = {"w1p": w1p, "b1p": b1f, "w2p": w2p, "b2p": b2}

    in_maps = []
    for core in range(NCORES):
        b, r = divmod(core, NCORES // B)
        rows = slice(TOK * r, TOK * (r + 1))
        # K dim-major, head-paired: [NPAIR, 128, KC, 128]
        kpa = np.ascontiguousarray(
            image_k[b].reshape(NPAIR, 2, KC, 128, HD)
            .transpose(0, 1, 4, 2, 3).reshape(NPAIR, 128, KC, 128)
            .astype(bf16))
        # Q dim-major, head-paired, scale folded: [NPAIR, 128, TOK]
        qpa = np.ascontiguousarray(
            (image_q[b, :, rows] * SCALE).reshape(NPAIR, 2, TOK, HD)
            .transpose(0, 1, 3, 2).reshape(NPAIR, 128, TOK)
            .astype(bf16))
        # V token-major per chunk with ones column: [NPAIR, 2, 128, KC, 65]
        vv = image_v[b].reshape(NPAIR, 2, KC, 128, HD).transpose(0, 1, 3, 2, 4)
        vpa = np.empty((NPAIR, 2, 128, KC, HD + 1), np.float16)
        vpa[..., :HD] = vv
        vpa[..., HD] = 1.0
        # x feature-major: [128, DC, TOK]
        xta = np.ascontiguousarray(
            x[b, rows].T.reshape(DC, 128, TOK).transpose(1, 0, 2))
        in_maps.append({
            "kp": kpa, "qp": qpa, "vp": vpa, "xt": xta, **shared,
        })
    return in_maps


def run_cores(in_maps, trace=False, **kw):
    nc = get_program()
    return run_bass_kernel_spmd(nc, in_maps, core_ids=list(range(NCORES)),
                                trace=trace, **kw)


def kernel(x, image_q, image_k, image_v, ln_w, ln_b, w1, b1, w2, b2):
    in_maps = make_in_maps(x, image_q, image_k, image_v, ln_w, ln_b,
                           w1, b1, w2, b2)
    res = run_cores(in_maps)
    out = np.empty((B, N, D), dtype=np.float32)
    for core in range(NCORES):
        b, r = divmod(core, NCORES // B)
        out[b, TOK * r:TOK * (r + 1)] = \
            np.asarray(res.results[core]["y"]).reshape(D, TOK).T
    return out
